# revision 7
# baseline (speedup 1.0000x reference)
"""MAP loss (per-pixel 3x3 Gaussian NLL) Trainium2 kernel.

loss = mean_{b,m,n}( 0.5*T' Sy^{-1} T + 0.5*log det Sy ),  T = (target-mu)[b,:,m,n]
with loss zeroed if max(0.5*T'Sy^{-1}T) > 1e7.

Sharding: pure data-parallel over the batch dim (16 batches -> 2 per core on
8 cores); each core returns [128, 36] partials which the host folds into the
scalar loss.

Per pixel: LDL' factorization of the symmetric 3x3, reformulated so every
pivot reciprocal is a fused custom-DVE op and no divisions/dets are needed:

    r0 = 1/s00            l10 = s01*r0   l20 = s02*r0   m1 = l10*s01
    r1 = 1/(s11 - m1)     f1  = s12 - l10*s02           l21 = f1*r1
    r2 = 1/(s22 - (l20*s02 + l21*f1))
    z1 = T1 - l10*T0      z2 = (T2 - l20*T0) - l21*z1
    t1 = 0.5*(T0^2*r0 + z1^2*r1 + z2^2*r2)
    logdet = -(ln r0 + ln r1 + ln r2)

v3 design (lessons from v1 exec=127us and v2 exec=164us traces):

* tg/mu land ONCE as full-resident bf16 SWDGE cast-DMAs (48KB/partition)
  -- no compute-coupled ring for the residual side, so the substitution
  chain is paced only by arrival order.
* sigma stays FP32 on the HWDGE (sync) queue in a 2-slot ring: bf16
  stride-9 reads are 2B-misaligned every other element and measured 2x
  slow on both ACT and DVE (v2), while fp32 36B-stride reads run at full
  rate (v1).  The two DMA queues drain concurrently at the HBM cap.
* GpSimd does NOTHING but SWDGE descriptor generation: v2 proved pool
  tensor ops hold the SBUF shared port pair for their full duration and
  starve every concurrent Vector tensor_tensor (2.6us vs 0.68us).
* All squares are stock bf16 tensor_tensor (2x) reusing already computed
  products (m1=l10*s01 etc.); customs are only the 3 pivot reciprocals
  (stride-9 fp32 diagonal reads) + 3 fused q-fold reductions (1x).
* s01/s02/s12 are deinterleaved to dense bf16 planes by Scalar (1.2us
  per 1024, v1-measured); the tail tiles shrink (1024x3/512/256/256) so
  the post-last-DMA drain is only the last 256-px substitution chain.
"""

import functools
import numpy as np

B, C, M, N = 16, 3, 512, 512
NCORES = 8
BS = B // NCORES          # batches per core
P = 128                   # SBUF partitions
QB = (M * N) // P         # pixels per partition per batch image (2048)
# compute tiles (batch, col offset, width): full-size through the bulk,
# shrinking at the end so the post-last-DMA drain is tiny.
TILES = [(0, 0, 1024), (0, 1024, 1024), (1, 0, 1024),
         (1, 1024, 512), (1, 1536, 256), (1, 1792, 256)]
NT = len(TILES)
NPIX = B * M * N
T1_CLIP = 1e7

_REGISTERED = {}


def _register_dve_ops():
    """Register the fused custom-DVE ops (idempotent). Uses the documented
    extension point: append to dve_ops.OPS / _SUB_OPCODE_FOR_NAME, with
    uops_sha computed in-process (self-consistent by construction)."""
    if _REGISTERED:
        return _REGISTERED
    from operator import add
    from concourse.dve_spec import (
        Spec, Src0, Src1, C0, C1, C2, Bin, AluOp, sq, lower, _has_src1,
    )
    from concourse.dve_uop import DveOpSpec
    import concourse.dve_ops as dops

    def make(name, spec, subdim=False):
        if name in dops._SUB_OPCODE_FOR_NAME:
            return next(o for o in dops.OPS if o.name == name)
        row = len(dops.OPS) + dops._CUSTOM_DVE_ROW_BASE
        dops._SUB_OPCODE_FOR_NAME[name] = row
        shas = {}
        for ver in ("v3", "v4"):
            s = DveOpSpec(name=name, opcode=row, uops=lower(spec, ver=ver),
                          rd1_en=_has_src1(spec))
            shas[ver] = s.sha(ver)
        op = dops.DveOp(name, spec, subdim=subdim, uops_sha=shas)
        dops.OPS.append(op)
        dops.CUSTOM_DVE_SPECS[name] = spec
        return op

    # out = approx 1/(in0 - in1): bitwise-NOT exponent-flip seed + one
    # Newton pass, computed on the internal fp32 stream so bf16 inputs are
    # fine. Constants are the stock minimax pair (optimal for 1-NR too).
    _d = Src0 - Src1
    _nd = Bin(AluOp.BITWISE_NOT, _d, _d)
    _y0 = _nd * C0

    def _ref_recip_sub(in0, in1, c0, c1, c2):
        d = in0.astype(np.float32) - in1.astype(np.float32)
        nd = (~d.view(np.int32)).view(np.float32)
        y0 = nd * c0
        return y0 * (c1 - d * y0)

    _REGISTERED["recip_sub"] = make(
        "RECIP_SUB_NR1_MAP",
        Spec(body=_y0 * (C1 - _d * _y0), reference=_ref_recip_sub),
    )
    # accum_out = sum(in0^2 * in1 * imm2): the whole 0.5*x^2*r quadratic
    # term, squared+scaled+folded in one 1x pass
    _REGISTERED["q_acc"] = make(
        "SQ_MUL_RED_MAP",
        Spec(body=sq(Src0) * Src1 * C2, accum=add,
             reference=lambda in0, in1, c0, c1, c2:
             in0.astype(np.float32) ** 2 * in1.astype(np.float32) * c2),
    )
    _REGISTERED["recip"] = dops.RECIPROCAL_APPROX_FAST
    _REGISTERED["consts"] = dops.RECIP_APPROX_FAST_CONSTS
    return _REGISTERED


def _emit_body(nc, tc, tgt, mu, sig, out):
    from concourse import mybir

    ops = _register_dve_ops()
    RC = ops["consts"]
    f32 = mybir.dt.float32
    bf16 = mybir.dt.bfloat16
    AF = mybir.ActivationFunctionType
    v = nc.vector
    sc = nc.scalar
    gp = nc.gpsimd

    with (
        tc.tile_pool(name="io", bufs=1) as iop,
        tc.tile_pool(name="wk", bufs=1) as wk,
        tc.tile_pool(name="acc", bufs=1) as accp,
    ):
        qld = accp.tile([P, 6 * NT], f32, tag="qld", bufs=1, name="qld")

        sig_f = sig.rearrange("b m n c d -> b (m n c d)")
        tgt_f = tgt.rearrange("b c m n -> b c (m n)")
        mu_f = mu.rearrange("b c m n -> b c (m n)")

        # ---- tg/mu: static bf16 tiles (one per compute tile, no rings);
        # sigma: fp32 ring of 2 slots on the HWDGE queue
        sig_t, tg_t, mu_t = [], [], []
        for ti, (b, o, w) in enumerate(TILES):
            tg_t.append(iop.tile([P, 3 * w], bf16, tag=f"tg{ti}", bufs=1,
                                 name=f"tg{ti}"))
            mu_t.append(iop.tile([P, 3 * w], bf16, tag=f"mv{ti}", bufs=1,
                                 name=f"mv{ti}"))

        def dma_sig(ti):
            b, o, w = TILES[ti]
            st = iop.tile([P, 9 * 1024], f32, tag="sig", bufs=2, name="sig")
            sig_t.append(st)
            src = sig_f[b].rearrange("(p q) -> p q", p=P)
            nc.sync.dma_start(out=st[:, :9 * w],
                              in_=src[:, o * 9:(o + w) * 9])

        def dma_tm(ti):
            b, o, w = TILES[ti]
            for dst, t in ((tg_t[ti], tgt_f), (mu_t[ti], mu_f)):
                src = t[b].rearrange("c (p q) -> p c q", p=P)
                gp.dma_start(
                    out=dst[:].rearrange("p (c q) -> p c q", c=3),
                    in_=src[:, :, o:o + w],
                )

        # ---- warm the ACT function-table set (copy/ln) during the fill
        pre = accp.tile([P, 1], f32, tag="pre", bufs=1, name="pre")
        pre2 = accp.tile([P, 1], f32, tag="pre2", bufs=1, name="pre2")
        v.memset(pre[:], 1.0)
        sc.activation(pre2[:], pre[:], AF.Ln)

        # ---- all input DMAs up front: sigma FIFO on sync (self-throttled
        # by the 2-slot ring), tg/mu FIFO on the SWDGE queue
        for ti in range(NT):
            dma_sig(ti)
        for ti in range(NT):
            dma_tm(ti)

        def bt(tag, bufs=1):
            return wk.tile([P, 1024], bf16, tag=tag, bufs=bufs, name=tag)

        def emit_ldl(ti):
            """Scalar deint of the multi-use off-diagonals + the vector
            LDL chain; diagonals stream stride-9 from the sigma tile."""
            b, o, w = TILES[ti]
            sv = sig_t[ti][:, :9 * w].rearrange("p (f k) -> p f k", k=9)
            st = {}
            for key, j in (("s01", 1), ("s02", 2), ("s12", 5)):
                pl = bt(key, bufs=2)
                sc.copy(pl[:, :w], sv[:, :, j])
                st[key] = pl[:, :w]

            def nt(tag, bufs=1):
                return bt(tag, bufs=bufs)[:, :w]

            s01, s02, s12 = st["s01"], st["s02"], st["s12"]
            r0 = nt("r0", bufs=2)
            v._custom_dve(ops["recip"], out=r0, in0=sv[:, :, 0],
                          s0=RC["s0"], s1=RC["s1"], imm2=RC["imm2"])
            l10 = nt("l10")
            v.tensor_mul(l10, s01, r0)
            m1 = nt("m1")
            v.tensor_mul(m1, l10, s01)
            r1 = nt("r1", bufs=2)
            v._custom_dve(ops["recip_sub"], out=r1, in0=sv[:, :, 4],
                          in1=m1, s0=RC["s0"], s1=RC["s1"])
            l20 = nt("l20")
            v.tensor_mul(l20, s02, r0)
            m2 = nt("m2")
            v.tensor_mul(m2, l10, s02)
            f1 = nt("f1")
            v.tensor_sub(f1, s12, m2)
            l21 = nt("l21")
            v.tensor_mul(l21, f1, r1)
            m3 = nt("m3")
            v.tensor_mul(m3, l20, s02)
            m4 = nt("m4")
            v.tensor_mul(m4, l21, f1)
            e2 = nt("e2")
            v.tensor_add(e2, m3, m4)
            r2 = nt("r2", bufs=2)
            v._custom_dve(ops["recip_sub"], out=r2, in0=sv[:, :, 8],
                          in1=e2, s0=RC["s0"], s1=RC["s1"])

            # Scalar: logdet contribution ln(r_i), summed along the free
            # dim into per-tile columns (sign folded on host)
            lnscr = nt("lnscr", bufs=2)
            for i, r in enumerate((r0, r1, r2)):
                sc.activation(lnscr, r, AF.Ln,
                              accum_out=qld[:, 3 * NT + 3 * ti + i:
                                            3 * NT + 3 * ti + i + 1])
            st.update(r0=r0, r1=r1, r2=r2, l10=l10, l20=l20, l21=l21)
            return st

        def emit_rest(ti, st):
            b, o, w = TILES[ti]

            def nt(tag, bufs=1):
                return bt(tag, bufs=bufs)[:, :w]

            tg3 = tg_t[ti][:].rearrange("p (c q) -> p c q", c=3)
            mu3 = mu_t[ti][:].rearrange("p (c q) -> p c q", c=3)
            T = []
            for c in range(3):
                Tc = nt(f"T{c}", bufs=2)
                v.tensor_sub(Tc, tg3[:, c], mu3[:, c])
                T.append(Tc)

            l10, l20, l21 = st["l10"], st["l20"], st["l21"]
            r0, r1, r2 = st["r0"], st["r1"], st["r2"]
            m5 = nt("m5")
            v.tensor_mul(m5, l10, T[0])
            z1 = nt("z1")
            v.tensor_sub(z1, T[1], m5)
            m6 = nt("m6")
            v.tensor_mul(m6, l20, T[0])
            h0 = nt("h0")
            v.tensor_sub(h0, T[2], m6)
            m7 = nt("m7")
            v.tensor_mul(m7, l21, z1)
            z2 = nt("z2")
            v.tensor_sub(z2, h0, m7)

            qscr = nt("qscr", bufs=2)
            for i, (x, r) in enumerate(((T[0], r0), (z1, r1), (z2, r2))):
                v._custom_dve(
                    ops["q_acc"], out=qscr, in0=x, in1=r, imm2=0.5,
                    accum_out=qld[:, 3 * ti + i:3 * ti + i + 1],
                )

        # software-pipelined emission: LDL(ti+1) ahead of rest(ti) so the
        # vector queue never head-of-line blocks on late tg/mu arrivals
        sts = [emit_ldl(0)]
        for ti in range(NT):
            if ti + 1 < NT:
                sts.append(emit_ldl(ti + 1))
            emit_rest(ti, sts[ti])

        nc.sync.dma_start(out=out[:, :], in_=qld[:])


@functools.lru_cache(maxsize=1)
def _build():
    import concourse.bacc as bacc
    import concourse.tile as tile
    from concourse import mybir

    _register_dve_ops()
    f32 = mybir.dt.float32
    nc = bacc.Bacc("TRN2", target_bir_lowering=False, debug=False)
    tgt = nc.dram_tensor("target_s", [BS, C, M, N], f32, kind="ExternalInput").ap()
    mu = nc.dram_tensor("mu_s", [BS, C, M, N], f32, kind="ExternalInput").ap()
    sig = nc.dram_tensor("sigma_s", [BS, M, N, C, C], f32, kind="ExternalInput").ap()
    out = nc.dram_tensor("partials", [P, 6 * NT], f32, kind="ExternalOutput").ap()
    with tile.TileContext(nc) as tc:
        _emit_body(nc, tc, tgt, mu, sig, out)
    nc.compile()
    return nc


def _run_on_device(target, mu, sigma_y, trace=False):
    from concourse.bass_utils import run_bass_kernel_spmd

    nc = _build()
    target = np.ascontiguousarray(target, dtype=np.float32)
    mu = np.ascontiguousarray(mu, dtype=np.float32)
    sigma_y = np.ascontiguousarray(sigma_y, dtype=np.float32)
    in_maps = [
        {
            "target_s": target[i * BS:(i + 1) * BS],
            "mu_s": mu[i * BS:(i + 1) * BS],
            "sigma_s": sigma_y[i * BS:(i + 1) * BS],
        }
        for i in range(NCORES)
    ]
    return run_bass_kernel_spmd(nc, in_maps, list(range(NCORES)), trace=trace)


def kernel(target, mu, sigma_mu, sigma_n, sigma_y):
    res = _run_on_device(target, mu, sigma_y)
    partials = [res.results[i]["partials"] for i in range(NCORES)]
    sum_q = sum(p[:, 0:3 * NT].astype(np.float64).sum() for p in partials)
    sum_lr = sum(p[:, 3 * NT:6 * NT].astype(np.float64).sum() for p in partials)
    # per-(tile,partition) q sums bound max(t1) since every q term >= 0
    bound = max(
        p[:, 3 * ti:3 * ti + 3].astype(np.float64).sum(axis=1).max()
        for p in partials for ti in range(NT)
    )
    loss = np.float32((sum_q - 0.5 * sum_lr) / NPIX)
    if bound > T1_CLIP:
        # Upper bound tripped: pay for the exact host-side check.
        t = np.transpose(
            (target - mu).astype(np.float64), (0, 2, 3, 1)
        )[..., :, None]
        sol = np.linalg.solve(sigma_y.astype(np.float64), t)
        t1 = 0.5 * np.einsum("bmnci,bmnci->bmn", t, sol)
        if t1.max() > T1_CLIP:
            loss = np.float32(0.0)
    return loss


# revision 15
# speedup vs baseline: 1.2682x; 1.2682x over previous
"""MAP loss (per-pixel 3x3 Gaussian NLL) Trainium2 kernel.

loss = mean_{b,m,n}( 0.5*T' Sy^{-1} T + 0.5*log det Sy ),  T = (target-mu)[b,:,m,n]
with loss zeroed if max(0.5*T'Sy^{-1}T) > 1e7.

Sharding: pure data-parallel over the batch dim (16 batches -> 2 per core on
8 cores); each core returns [128, 6*NT] partials which the host folds into
the scalar loss.

Per pixel: LDL' factorization of the symmetric 3x3, with every pivot
reciprocal a fused custom-DVE op so no divisions/dets are needed:

    r0 = 1/s00            l10 = s01*r0   l20 = s02*r0   m1 = l10*s01
    r1 = 1/(s11 - m1)     f1  = s12 - l10*s02           l21 = f1*r1
    r2 = 1/(s22 - (l20*s02 + l21*f1))
    z1 = T1 - l10*T0      z2 = (T2 - l20*T0) - l21*z1
    t1 = 0.5*(T0^2*r0 + z1^2*r1 + z2^2*r2)
    logdet = -(ln r0 + ln r1 + ln r2)

v4 design (HW-measured costs from the v1/v2/v3 traces and probes):

* ALL inputs land once as full-resident bf16 SWDGE cast-DMAs on a single
  FIFO queue (sigma 72KB/partition, tg/mu 48KB) issued in compute order
  with sigma one chunk ahead -- v2 proved this drains wall-to-wall at
  ~342 GB/s with zero mid-kernel stalls (31.5MB ~ 92us).
* GpSimd does NOTHING but descriptor generation: pool tensor ops hold the
  SBUF shared port pair and starve Vector's tensor_tensor src1 port
  (measured 2.6us vs 0.68us per TT in v2).
* Stride-9 reads cost ~2 cyc/elem on BOTH ACT and DVE (any dtype), so all
  six sigma entries are deinterleaved by Scalar (6 x 2.1us per 1024) into
  dense bf16 planes; every Vector op then runs dense (TT 2x, customs 1x).
* r0/r1/r2 are written into thirds of ONE [P,3w] tile so the logdet is a
  single Ln+accum per tile instead of three.
* Tiles 512/512/1024 + 1024/512/256/256: small first tile shortens the
  pipeline fill, small last tiles shorten the post-DMA drain.
"""

import functools
import numpy as np

B, C, M, N = 16, 3, 512, 512
NCORES = 8
BS = B // NCORES          # batches per core
P = 128                   # SBUF partitions
QB = (M * N) // P         # pixels per partition per batch image (2048)
TILES = [(0, 0, 512), (0, 512, 512), (0, 1024, 1024),
         (1, 0, 1024), (1, 1024, 512), (1, 1536, 256), (1, 1792, 256)]
NT = len(TILES)
NPIX = B * M * N
T1_CLIP = 1e7

_REGISTERED = {}


def _register_dve_ops():
    """Register the fused custom-DVE ops (idempotent). Uses the documented
    extension point: append to dve_ops.OPS / _SUB_OPCODE_FOR_NAME, with
    uops_sha computed in-process (self-consistent by construction)."""
    if _REGISTERED:
        return _REGISTERED
    from operator import add
    from concourse.dve_spec import (
        Spec, Src0, Src1, C0, C1, C2, Bin, AluOp, sq, lower, _has_src1,
    )
    from concourse.dve_uop import DveOpSpec
    import concourse.dve_ops as dops

    def make(name, spec, subdim=False):
        if name in dops._SUB_OPCODE_FOR_NAME:
            return next(o for o in dops.OPS if o.name == name)
        row = len(dops.OPS) + dops._CUSTOM_DVE_ROW_BASE
        dops._SUB_OPCODE_FOR_NAME[name] = row
        shas = {}
        for ver in ("v3", "v4"):
            s = DveOpSpec(name=name, opcode=row, uops=lower(spec, ver=ver),
                          rd1_en=_has_src1(spec))
            shas[ver] = s.sha(ver)
        op = dops.DveOp(name, spec, subdim=subdim, uops_sha=shas)
        dops.OPS.append(op)
        dops.CUSTOM_DVE_SPECS[name] = spec
        return op

    # out = approx 1/(in0 - in1): bitwise-NOT exponent-flip seed + one
    # Newton pass, computed on the internal fp32 stream so bf16 inputs are
    # fine. Constants are the stock minimax pair (optimal for 1-NR too).
    _d = Src0 - Src1
    _nd = Bin(AluOp.BITWISE_NOT, _d, _d)
    _y0 = _nd * C0

    def _ref_recip_sub(in0, in1, c0, c1, c2):
        d = in0.astype(np.float32) - in1.astype(np.float32)
        nd = (~d.view(np.int32)).view(np.float32)
        y0 = nd * c0
        return y0 * (c1 - d * y0)

    _REGISTERED["recip_sub"] = make(
        "RECIP_SUB_NR1_MAP",
        Spec(body=_y0 * (C1 - _d * _y0), reference=_ref_recip_sub),
    )
    # accum_out = sum(in0^2 * in1 * imm2): the whole 0.5*x^2*r quadratic
    # term, squared+scaled+folded in one 1x pass
    _REGISTERED["q_acc"] = make(
        "SQ_MUL_RED_MAP",
        Spec(body=sq(Src0) * Src1 * C2, accum=add,
             reference=lambda in0, in1, c0, c1, c2:
             in0.astype(np.float32) ** 2 * in1.astype(np.float32) * c2),
    )
    _REGISTERED["recip"] = dops.RECIPROCAL_APPROX_FAST
    _REGISTERED["consts"] = dops.RECIP_APPROX_FAST_CONSTS
    return _REGISTERED


def _emit_body(nc, tc, tgt, mu, sig, out):
    from concourse import mybir

    ops = _register_dve_ops()
    RC = ops["consts"]
    f32 = mybir.dt.float32
    bf16 = mybir.dt.bfloat16
    AF = mybir.ActivationFunctionType
    v = nc.vector
    sc = nc.scalar
    gp = nc.gpsimd

    with (
        tc.tile_pool(name="io", bufs=1) as iop,
        tc.tile_pool(name="wk", bufs=1) as wk,
        tc.tile_pool(name="acc", bufs=1) as accp,
    ):
        # cols [0:3*NT) = q terms, [3*NT:6*NT) = logdet sums (full-width
        # tiles write one fused column and leave their other two zeroed)
        qld = accp.tile([P, 6 * NT], f32, tag="qld", bufs=1, name="qld")

        sig_f = sig.rearrange("b m n c d -> b (m n c d)")
        tgt_f = tgt.rearrange("b c m n -> b c (m n)")
        mu_f = mu.rearrange("b c m n -> b c (m n)")

        # ---- static bf16 input tiles, one per compute tile, no rings
        sig_t, tg_t, mu_t = [], [], []
        for ti, (b, o, w) in enumerate(TILES):
            sig_t.append(iop.tile([P, 9 * w], bf16, tag=f"sg{ti}", bufs=1,
                                  name=f"sg{ti}"))
            tg_t.append(iop.tile([P, 3 * w], bf16, tag=f"tg{ti}", bufs=1,
                                 name=f"tg{ti}"))
            mu_t.append(iop.tile([P, 3 * w], bf16, tag=f"mv{ti}", bufs=1,
                                 name=f"mv{ti}"))

        def dma_sig(ti):
            b, o, w = TILES[ti]
            src = sig_f[b].rearrange("(p q) -> p q", p=P)
            gp.dma_start(out=sig_t[ti][:], in_=src[:, o * 9:(o + w) * 9])

        def dma_tm(ti):
            b, o, w = TILES[ti]
            for dst, t in ((tg_t[ti], tgt_f), (mu_t[ti], mu_f)):
                src = t[b].rearrange("c (p q) -> p c q", p=P)
                gp.dma_start(
                    out=dst[:].rearrange("p (c q) -> p c q", c=3),
                    in_=src[:, :, o:o + w],
                )

        # ---- warm the ACT function-table set (copy/ln) during the fill
        pre = accp.tile([P, 1], f32, tag="pre", bufs=1, name="pre")
        pre2 = accp.tile([P, 1], f32, tag="pre2", bufs=1, name="pre2")
        v.memset(pre[:], 1.0)
        v.memset(qld[:], 0.0)
        sc.activation(pre2[:], pre[:], AF.Ln)

        # ---- all input DMAs up front: single SWDGE FIFO queue in compute
        # order, sigma one chunk ahead (it gates the longer chain)
        dma_sig(0)
        dma_sig(1)
        dma_tm(0)
        dma_sig(2)
        dma_tm(1)
        dma_sig(3)
        dma_tm(2)
        dma_sig(4)
        dma_tm(3)
        dma_sig(5)
        dma_tm(4)
        dma_sig(6)
        dma_tm(5)
        dma_tm(6)

        def bt(tag, bufs=1, mult=1):
            return wk.tile([P, mult * 1024], bf16, tag=tag, bufs=bufs,
                           name=tag)

        def emit_ldl(ti):
            """Scalar deint of all six sigma entries into dense bf16
            planes + the vector LDL chain (everything dense)."""
            b, o, w = TILES[ti]
            sv = sig_t[ti][:].rearrange("p (f k) -> p f k", k=9)
            st = {}
            for key, j in (("s00", 0), ("s01", 1), ("s02", 2),
                           ("s11", 4), ("s12", 5), ("s22", 8)):
                pl = bt(key, bufs=2)
                sc.copy(pl[:, :w], sv[:, :, j])
                st[key] = pl[:, :w]

            def nt(tag, bufs=1):
                return bt(tag, bufs=bufs)[:, :w]

            s00, s01, s02 = st["s00"], st["s01"], st["s02"]
            s11, s12, s22 = st["s11"], st["s12"], st["s22"]
            # r0/r1/r2 live in thirds of one tile so logdet is 1 Ln+accum
            rr = bt("rr", bufs=2, mult=3)
            r0, r1, r2 = rr[:, 0:w], rr[:, 1024:1024 + w], rr[:, 2048:2048 + w]
            v._custom_dve(ops["recip"], out=r0, in0=s00,
                          s0=RC["s0"], s1=RC["s1"], imm2=RC["imm2"])
            l10 = nt("l10")
            v.tensor_mul(l10, s01, r0)
            m1 = nt("m1")
            v.tensor_mul(m1, l10, s01)
            v._custom_dve(ops["recip_sub"], out=r1, in0=s11,
                          in1=m1, s0=RC["s0"], s1=RC["s1"])
            l20 = nt("l20")
            v.tensor_mul(l20, s02, r0)
            m2 = nt("m2")
            v.tensor_mul(m2, l10, s02)
            f1 = nt("f1")
            v.tensor_sub(f1, s12, m2)
            l21 = nt("l21")
            v.tensor_mul(l21, f1, r1)
            m3 = nt("m3")
            v.tensor_mul(m3, l20, s02)
            m4 = nt("m4")
            v.tensor_mul(m4, l21, f1)
            e2 = nt("e2")
            v.tensor_add(e2, m3, m4)
            v._custom_dve(ops["recip_sub"], out=r2, in0=s22,
                          in1=e2, s0=RC["s0"], s1=RC["s1"])

            # logdet contribution via Ln+accum (sign folded on host): one
            # fused op over the contiguous [r0|r1|r2] when w==1024, else
            # one op per third, each into its own column.
            lnscr = bt("lnscr", bufs=1, mult=3)
            if w == 1024:
                sc.activation(lnscr[:, :3 * 1024], rr[:, :3 * 1024], AF.Ln,
                              accum_out=qld[:, 3 * NT + 3 * ti:
                                            3 * NT + 3 * ti + 1])
            else:
                for i, r in enumerate((r0, r1, r2)):
                    sc.activation(lnscr[:, :w], r, AF.Ln,
                                  accum_out=qld[:, 3 * NT + 3 * ti + i:
                                                3 * NT + 3 * ti + i + 1])
            st.update(r0=r0, r1=r1, r2=r2, l10=l10, l20=l20, l21=l21)
            return st

        def emit_rest(ti, st):
            b, o, w = TILES[ti]

            def nt(tag, bufs=1):
                return bt(tag, bufs=bufs)[:, :w]

            tg3 = tg_t[ti][:].rearrange("p (c q) -> p c q", c=3)
            mu3 = mu_t[ti][:].rearrange("p (c q) -> p c q", c=3)
            T = []
            for c in range(3):
                Tc = nt(f"T{c}", bufs=2)
                v.tensor_sub(Tc, tg3[:, c], mu3[:, c])
                T.append(Tc)

            l10, l20, l21 = st["l10"], st["l20"], st["l21"]
            r0, r1, r2 = st["r0"], st["r1"], st["r2"]
            m5 = nt("m5")
            v.tensor_mul(m5, l10, T[0])
            z1 = nt("z1")
            v.tensor_sub(z1, T[1], m5)
            m6 = nt("m6")
            v.tensor_mul(m6, l20, T[0])
            h0 = nt("h0")
            v.tensor_sub(h0, T[2], m6)
            m7 = nt("m7")
            v.tensor_mul(m7, l21, z1)
            z2 = nt("z2")
            v.tensor_sub(z2, h0, m7)

            qscr = nt("qscr")
            for i, (x, r) in enumerate(((T[0], r0), (z1, r1), (z2, r2))):
                v._custom_dve(
                    ops["q_acc"], out=qscr, in0=x, in1=r, imm2=0.5,
                    accum_out=qld[:, 3 * ti + i:3 * ti + i + 1],
                )

        # software-pipelined emission: LDL(ti+1) ahead of rest(ti) so the
        # vector queue never head-of-line blocks on late tg/mu arrivals
        sts = [emit_ldl(0)]
        for ti in range(NT):
            if ti + 1 < NT:
                sts.append(emit_ldl(ti + 1))
            emit_rest(ti, sts[ti])

        nc.sync.dma_start(out=out[:, :], in_=qld[:])


@functools.lru_cache(maxsize=1)
def _build():
    import concourse.bacc as bacc
    import concourse.tile as tile
    from concourse import mybir

    _register_dve_ops()
    f32 = mybir.dt.float32
    nc = bacc.Bacc("TRN2", target_bir_lowering=False, debug=False)
    tgt = nc.dram_tensor("target_s", [BS, C, M, N], f32, kind="ExternalInput").ap()
    mu = nc.dram_tensor("mu_s", [BS, C, M, N], f32, kind="ExternalInput").ap()
    sig = nc.dram_tensor("sigma_s", [BS, M, N, C, C], f32, kind="ExternalInput").ap()
    out = nc.dram_tensor("partials", [P, 6 * NT], f32, kind="ExternalOutput").ap()
    with tile.TileContext(nc) as tc:
        _emit_body(nc, tc, tgt, mu, sig, out)
    nc.compile()
    return nc


def _run_on_device(target, mu, sigma_y, trace=False):
    from concourse.bass_utils import run_bass_kernel_spmd

    nc = _build()
    target = np.ascontiguousarray(target, dtype=np.float32)
    mu = np.ascontiguousarray(mu, dtype=np.float32)
    sigma_y = np.ascontiguousarray(sigma_y, dtype=np.float32)
    in_maps = [
        {
            "target_s": target[i * BS:(i + 1) * BS],
            "mu_s": mu[i * BS:(i + 1) * BS],
            "sigma_s": sigma_y[i * BS:(i + 1) * BS],
        }
        for i in range(NCORES)
    ]
    return run_bass_kernel_spmd(nc, in_maps, list(range(NCORES)), trace=trace)


def kernel(target, mu, sigma_mu, sigma_n, sigma_y):
    res = _run_on_device(target, mu, sigma_y)
    partials = [res.results[i]["partials"] for i in range(NCORES)]
    sum_q = sum(p[:, 0:3 * NT].astype(np.float64).sum() for p in partials)
    sum_lr = sum(p[:, 3 * NT:6 * NT].astype(np.float64).sum() for p in partials)
    # per-(tile,partition) q sums bound max(t1) since every q term >= 0
    bound = max(
        p[:, 3 * ti:3 * ti + 3].astype(np.float64).sum(axis=1).max()
        for p in partials for ti in range(NT)
    )
    loss = np.float32((sum_q - 0.5 * sum_lr) / NPIX)
    if bound > T1_CLIP:
        # Upper bound tripped: pay for the exact host-side check.
        t = np.transpose(
            (target - mu).astype(np.float64), (0, 2, 3, 1)
        )[..., :, None]
        sol = np.linalg.solve(sigma_y.astype(np.float64), t)
        t1 = 0.5 * np.einsum("bmnci,bmnci->bmn", t, sol)
        if t1.max() > T1_CLIP:
            loss = np.float32(0.0)
    return loss
